# revision 71
# baseline (speedup 1.0000x reference)
"""Trainium2 Bass kernel for nn_AttentionBlock (GroupNorm + 4-head attention + proj).

Sharding: 8 cores = (batch b in {0,1}) x (t-quarter tq in {0..3}).
Each core computes, for its batch and its 1024-wide query slice:
  - GroupNorm stats over the full [256, 4096] batch slab (bn_stats on fp8 x)
  - GN affine folded into the qkv weights on device (W' = W*A, bias' += W@B),
    so q/k/v are produced straight from raw fp8 x with fp8 DoubleRow matmuls
  - flash-style attention: QK^T in fp8e4m3 DoubleRow (second k-tile multiplied
    by a zero q-tile), softmax exp split across ACT (true Exp -> e5m2) and
    DVE/Pool (Schraudolph bit-trick exp: int8(s*A+B) bitcast as e5m2),
    denominator via a ones-column in the AV matmul
  - AV in fp8 DoubleRow over s-chunk pairs (v e4m3 x p e5m2)
  - proj in bf16; proj bias + residual fused in one scalar_tensor_tensor
Host only slices/casts inputs per core and concatenates the 8 output tiles.

Precision: scores fp32 in PSUM, all accumulation fp32, softmax weights e5m2
(exact-exp or Schraudolph within +-6%), v e4m3, a bf16.  Validated end-to-end
rel err ~6e-3 vs fp64 reference (gate 2e-2).
"""

import os
import sys

for _p in ("/opt/trn_rl_repo", "/opt/pypackages"):
    if _p not in sys.path and os.path.isdir(_p):
        sys.path.append(_p)

import ml_dtypes
import numpy as np

import concourse.tile as tile
from concourse import bacc, bass2jax, mybir

# ---------------- problem constants ----------------
B, C, HS, WS = 2, 256, 64, 64
T = HS * WS            # 4096
NH = 4                 # heads
CH = C // NH           # 64 channels / head
GROUPS = 32
GSIZE = C // GROUPS    # 8 channels / group
EPS = 1e-5
SCALE = CH ** -0.25

NCORES = 8
TQ = T // 4            # 1024 query positions per core
SB = 128               # s-block (key positions per QK matmul)
NSB = T // SB          # 32 s-blocks
NPAIR = NSB // 2       # 16 s-block pairs per unit
TT = 512               # t-tile width
NTT = TQ // TT         # 2 t-tiles per core

F32 = mybir.dt.float32
BF16 = mybir.dt.bfloat16
E4 = mybir.dt.float8e4
E5 = mybir.dt.float8e5
I8 = mybir.dt.int8
DR = mybir.MatmulPerfMode.DoubleRow

# Schraudolph exp -> e5m2 bits: i8 = trunc(s*A + B); bitcast e5m2.
# Valid for scores in (-10.4, +10.9); actual range is [-7.1, +7.1].
SCH_A = 4.0 / np.log(2.0)
SCH_B = 15 * 4 + 0.5   # +0.5 converts the convert-truncation into rounding

# exp engine schedule per unit: 16 s-block-pair exps across ACT/DVE.
# (GPSIMD/Pool cannot access PSUM on real hardware, so the softmax exp is
# split between the two PSUM-capable vector engines only; Pool gets the
# SBUF-to-SBUF side work.)
def _build_exp_pattern(na, nd):
    quota = {"act": na, "dve": nd}
    rate = {"act": 1053.0, "dve": 1192.0}
    used = {k: 0.0 for k in quota}
    out = []
    for _ in range(na + nd):
        pick = min((k for k in quota if quota[k] > 0),
                   key=lambda k: (used[k] + rate[k]))
        out.append(pick)
        used[pick] += rate[pick]
        quota[pick] -= 1
    return out

EXP_PATTERN = _build_exp_pattern(9, 7)
AV_LAG = 4             # pair-slots the AV matmul trails the exp by


def build_nc():
    nc = bacc.Bacc("TRN2", target_bir_lowering=False, debug=False)

    # ---- I/O ----
    # consts blob columns: gind[0:16] bqkv[16:22] pbias[22:24] normw[24:26] normb[26:28]
    x8_ext = nc.declare_dram_parameter("x8", [128, 2, T], I8, isOutput=False)
    xq8_ext = nc.declare_dram_parameter("xq8", [128, 2, TQ], I8, isOutput=False)
    xq_ext = nc.declare_dram_parameter("xq", [128, 2, TQ], F32, isOutput=False)
    wqkvT_ext = nc.declare_dram_parameter("wqkvT", [128, 2, 3 * C], BF16, isOutput=False)
    wprojT_ext = nc.declare_dram_parameter("wprojT", [128, 2, C], BF16, isOutput=False)
    consts_ext = nc.declare_dram_parameter("consts", [128, 28], F32, isOutput=False)
    gindT_ext = nc.declare_dram_parameter("gindT", [16, 128], F32, isOutput=False)
    out_ext = nc.declare_dram_parameter("out", [C, TQ], F32, isOutput=True)
    lscr = nc.dram_tensor("lscr", [NTT * NH, TT], F32)

    DBG = os.environ.get("KDBG") == "1"
    if DBG:
        dbg_k8 = nc.declare_dram_parameter("dbg_k8", [128, 2, T], E4, isOutput=True)
        dbg_q8 = nc.declare_dram_parameter("dbg_q8", [128, 2, TQ], E4, isOutput=True)
        dbg_vT = nc.declare_dram_parameter("dbg_vT", [128, NSB, NH, CH + 4], E4, isOutput=True)
        dbg_a = nc.declare_dram_parameter("dbg_a", [128, 2, TQ], BF16, isOutput=True)
        dbg_pt = nc.declare_dram_parameter("dbg_pt", [128, NSB, TT], E5, isOutput=True)
        dbg_ga = nc.declare_dram_parameter("dbg_ga", [128, 2, 2], F32, isOutput=True)
        dbg_ob = nc.declare_dram_parameter("dbg_ob", [128, 2, 1], F32, isOutput=True)
        dbg_bqk = nc.declare_dram_parameter("dbg_bqk", [128, 4, 1], F32, isOutput=True)
        dbg_vbx = nc.declare_dram_parameter("dbg_vbx", [128, 2, 1], BF16, isOutput=True)
        dbg_pwb = nc.declare_dram_parameter("dbg_pwb", [128, 6], F32, isOutput=True)

    with tile.TileContext(nc) as tc:
        with (
            tc.tile_pool(name="sing", bufs=1) as sing,
            tc.tile_pool(name="ptp", bufs=16) as ptp,
            tc.tile_pool(name="tmp", bufs=4) as tmp,
            tc.tile_pool(name="psQ", bufs=3, space="PSUM") as psQ,
            tc.tile_pool(name="psV", bufs=2, space="PSUM") as psV,
        ):
            # ---------------- persistent SBUF ----------------
            x8 = sing.tile([128, 2, T], E4)            # raw x, fp8 (stats+prod)
            xq8 = sing.tile([128, 2, TQ], E4)          # raw x quarter, fp8 (q)
            xq = sing.tile([128, 2, TQ], F32)          # raw x quarter (residual)
            w_qkv = sing.tile([128, 2, 3 * C], BF16)   # qkv weights (bf16)
            w_qkv8 = sing.tile([128, 2, 3 * C], E4)    # qkv weights * A (fp8)
            w_proj = sing.tile([128, 2, C], BF16)
            k8 = sing.tile([128, 2, T + SB], E4)       # k  [p, mb, s] (+pad)
            q8z = sing.tile([128, 2, 2, TQ], E4)       # q  [p, mb, {q,0}, t]
            vT = sing.tile([128, NSB, NH, CH + 4], E4)  # v^T [s, chunk, h, c|1|pad]
            # (per-head slot 68B so the DoubleRow chunk stride 4*68=272 is 16B-aligned)
            a_sb = sing.tile([128, 2, TQ], BF16)       # attention out (channels)
            out_sb = sing.tile([128, 2, TQ], F32)
            consts = sing.tile([128, 28], F32)         # packed constants
            gind = consts[:, 0:16]
            bqkv_sb = consts[:, 16:22]                 # q0,q1,k0,k1,v0,v1 bias
            pbias = consts[:, 22:24]                   # proj_b + Wp@bv (host)
            normw = consts[:, 24:26]
            normb = consts[:, 26:28]
            bias_qk = sing.tile([128, 4, 1], F32)      # bias incl. W@B shift
            vbx = sing.tile([128, 2, 1], BF16)         # Wv@B (v-channel shift)
            obias = sing.tile([128, 2, 1], F32)        # pbias + Wp@(Wv@B)
            gindT = sing.tile([16, 128], F32)
            eps16 = sing.tile([16, 1], F32)
            ga = sing.tile([128, 2, 2], F32)           # per-channel [A, B]
            gab = sing.tile([128, 2, 1], BF16)         # B in bf16 (for W@B)

            # ---------------- input / constant DMAs ----------------
            for chk in range(4):
                nc.sync.dma_start(
                    out=x8[:, :, chk * 1024: (chk + 1) * 1024].bitcast(I8),
                    in_=x8_ext[:, :, chk * 1024: (chk + 1) * 1024],
                )
            nc.sync.dma_start(out=consts, in_=consts_ext[:, :])
            nc.sync.dma_start(out=gindT, in_=gindT_ext[:, :])
            nc.sync.dma_start(out=w_qkv, in_=wqkvT_ext[:, :, :])
            nc.sync.dma_start(out=xq8.bitcast(I8), in_=xq8_ext[:, :, :])
            nc.sync.dma_start(out=w_proj, in_=wprojT_ext[:, :, :])
            nc.sync.dma_start(out=xq, in_=xq_ext[:, :, :])
            # constants on the (initially idle) Pool engine
            nc.gpsimd.memset(q8z[:, :, 1, :], 0.0)     # zero q-tiles (DoubleRow)
            nc.gpsimd.memset(vT[:, :, :, CH: CH + 1], 1.0)  # ones col (denom)
            nc.gpsimd.memset(k8[:, :, T:], 0.0)        # pad cols (NaN-safe)
            nc.vector.memset(eps16, EPS)

            # ---------------- GroupNorm stats (from fp8 x) ----------------
            # split across DVE (bn_stats, 6 chunks/cb) and ACT (sum + sum-of-
            # squares via accumulate, 2 chunks/cb) to shorten the serial
            # startup chain; merged below with 3/4 : 1/4 weights.
            ACT_CHUNKS = (0, 2)
            stats = tmp.tile([128, 2, 6, 6], F32, tag="bnst")
            asum = tmp.tile([128, 2, 2, 2], F32, tag="asum")
            ascr = tmp.tile([128, 512], F32, tag="ascr")
            for cb in range(2):
                for j, kk in enumerate(ACT_CHUNKS):
                    xs = x8[:, cb, kk * 512: (kk + 1) * 512]
                    nc.scalar.activation(
                        out=ascr, in_=xs,
                        func=mybir.ActivationFunctionType.Copy, scale=1.0,
                        accum_out=asum[:, cb, j, 0:1])
                    nc.scalar.activation(
                        out=ascr, in_=xs,
                        func=mybir.ActivationFunctionType.Square, scale=1.0,
                        accum_out=asum[:, cb, j, 1:2])
            for chk in range(4):                        # chase the x8 DMAs
                for cb in range(2):
                    for k2 in range(2):
                        kk = chk * 2 + k2
                        if kk in ACT_CHUNKS:
                            continue
                        di = kk - sum(1 for a in ACT_CHUNKS if a < kk)
                        nc.vector.bn_stats(
                            out=stats[:, cb, di, :],
                            in_=x8[:, cb, kk * 512: (kk + 1) * 512],
                        )
            for cb in range(2):
                mv6 = tmp.tile([128, 2], F32, tag="mv6")
                nc.vector.bn_aggr(out=mv6, in_=stats[:, cb, :, :])
                # E[x^2] over DVE chunks = var + mean^2
                msq = tmp.tile([128, 1], F32, tag="msq")
                nc.vector.tensor_mul(msq, mv6[:, 0:1], mv6[:, 0:1])
                nc.vector.tensor_add(mv6[:, 1:2], mv6[:, 1:2], msq)
                # merge: mv = 0.75*mv6 + (act_sums/4096)
                mv = tmp.tile([128, 2], F32, tag="mv")
                nc.vector.tensor_add(
                    mv, asum[:, cb, 0, :], asum[:, cb, 1, :])
                nc.vector.tensor_scalar(
                    out=mv, in0=mv, scalar1=1.0 / 4096.0, scalar2=None,
                    op0=mybir.AluOpType.mult)
                nc.vector.scalar_tensor_tensor(
                    out=mv, in0=mv6, scalar=0.75, in1=mv,
                    op0=mybir.AluOpType.mult, op1=mybir.AluOpType.add)
                # group-aggregate: [16, 2] = gind^T @ [mean_c, E[x^2]_c] (avg /8)
                pg = psQ.tile([128, 2, TT], F32, tag="qk")
                gstat = pg[0:16, 0, 0:2]
                nc.tensor.matmul(gstat, lhsT=gind, rhs=mv, start=True, stop=True)
                gs_s = tmp.tile([16, 2], F32, tag="gss")
                nc.vector.tensor_copy(gs_s, gstat)
                # var_g = E[x^2]_g - mean_g^2 ; rstd_g = 1/sqrt(var_g + eps)
                g_ms = tmp.tile([16, 1], F32, tag="gms")
                nc.vector.tensor_mul(g_ms, gs_s[:, 0:1], gs_s[:, 0:1])
                g_sr = tmp.tile([16, 2], F32, tag="gsr")  # [rstd_g, mean_g]
                nc.vector.tensor_sub(g_sr[:, 0:1], gs_s[:, 1:2], g_ms)
                nc.scalar.activation(
                    out=g_sr[:, 0:1], in_=g_sr[:, 0:1],
                    func=mybir.ActivationFunctionType.Sqrt,
                    bias=eps16, scale=1.0,
                )
                nc.vector.reciprocal(g_sr[:, 0:1], g_sr[:, 0:1])
                nc.vector.tensor_copy(g_sr[:, 1:2], gs_s[:, 0:1])
                # broadcast group->channel via matmul with indicator
                pc = psQ.tile([128, 2, TT], F32, tag="qk")
                cstat = pc[:, 0, 0:2]
                nc.tensor.matmul(cstat, lhsT=gindT, rhs=g_sr, start=True, stop=True)
                # A = rstd*w ; Bb = normb - mean*A
                nc.vector.tensor_mul(ga[:, cb, 0:1], cstat[:, 0:1], normw[:, cb:cb+1])
                mA = tmp.tile([128, 1], F32, tag="mA")
                nc.vector.tensor_mul(mA, cstat[:, 1:2], ga[:, cb, 0:1])
                nc.vector.tensor_sub(ga[:, cb, 1:2], normb[:, cb:cb+1], mA)
                nc.vector.tensor_copy(gab[:, cb, :], ga[:, cb, 1:2])

            # ---------------- fold GN affine into weights ----------------
            # W' = W * A_c (rows scaled, fp8 out); split across DVE / Pool
            nc.gpsimd.tensor_scalar(
                out=w_qkv8[:, 0, :], in0=w_qkv[:, 0, :],
                scalar1=ga[:, 0, 0:1], scalar2=None, op0=mybir.AluOpType.mult,
            )
            nc.gpsimd.tensor_scalar(
                out=w_qkv8[:, 1, :], in0=w_qkv[:, 1, :],
                scalar1=ga[:, 1, 0:1], scalar2=None, op0=mybir.AluOpType.mult,
            )
            # W@B per output chunk (q0,q1,k0,k1,v0,v1): N=1 matmuls
            pwb = psQ.tile([128, 2, TT], F32, tag="qk")
            for j in range(6):
                for cb in range(2):
                    nc.tensor.matmul(
                        pwb[:, 0, j: j + 1],
                        lhsT=w_qkv[:, cb, j * 128: (j + 1) * 128],
                        rhs=gab[:, cb, :],
                        start=(cb == 0), stop=(cb == 1),
                    )
            # q/k production bias = bqkv + W@B ; v shift -> vbx (bf16)
            nc.vector.tensor_add(
                bias_qk.rearrange("p j one -> p (j one)"),
                bqkv_sb[:, 0:4],
                pwb[:, 0, 0:4],
            )
            # v shift is W@B only: host already folded bv into pbias
            nc.vector.tensor_copy(vbx.rearrange("p j one -> p (j one)"),
                                  pwb[:, 0, 4:6])
            # obias = pbias + Wp @ vbx
            pob = psQ.tile([128, 2, TT], F32, tag="qk")
            for mb in range(2):
                for cb in range(2):
                    nc.tensor.matmul(
                        pob[:, 0, mb: mb + 1],
                        lhsT=w_proj[:, cb, mb * 128: (mb + 1) * 128],
                        rhs=vbx[:, cb, :],
                        start=(cb == 0), stop=(cb == 1),
                    )
            nc.vector.tensor_add(
                obias.rearrange("p j one -> p (j one)"),
                pbias,
                pob[:, 0, 0:2],
            )
            if DBG:
                nc.sync.dma_start(out=dbg_ob[:, :, :], in_=obias)
                nc.sync.dma_start(out=dbg_bqk[:, :, :], in_=bias_qk)
                nc.sync.dma_start(out=dbg_vbx[:, :, :], in_=vbx)
                dbg_pwb_sb = tmp.tile([128, 6], F32, tag="dbgp")
                nc.vector.tensor_copy(dbg_pwb_sb, pwb[:, 0, 0:6])
                nc.sync.dma_start(out=dbg_pwb[:, :], in_=dbg_pwb_sb)

            # ---------------- q/k/v production (fp8 DoubleRow) ----------
            ecnt = [0]

            def evict(dst, src, bias=None):
                e = ("act", "dve")[ecnt[0] % 2]
                ecnt[0] += 1
                if e == "act":
                    if bias is None:
                        nc.scalar.activation(
                            out=dst, in_=src,
                            func=mybir.ActivationFunctionType.Copy, scale=1.0)
                    else:
                        nc.scalar.activation(
                            out=dst, in_=src,
                            func=mybir.ActivationFunctionType.Identity,
                            bias=bias, scale=1.0)
                else:
                    if bias is None:
                        nc.vector.tensor_copy(dst, src)
                    else:
                        nc.vector.tensor_scalar(out=dst, in0=src, scalar1=bias,
                                                scalar2=None,
                                                op0=mybir.AluOpType.add)

            def emit_q(mb):
                pq = psQ.tile([128, 2, TT], F32, tag="qk")
                for jt in range(2):
                    nc.tensor.matmul(
                        pq[:, jt, :],
                        lhsT=w_qkv8[:, :, mb * 128: (mb + 1) * 128],
                        rhs=xq8[:, :, jt * TT: (jt + 1) * TT],
                        start=True, stop=True, perf_mode=DR,
                    )
                evict(q8z[:, mb, 0, :].rearrange("p (j t) -> p j t", j=2),
                      pq, bias_qk[:, mb, :])

            def emit_k(mb, ck):
                pk = psQ.tile([128, 2, TT], F32, tag="qk")
                for jt in range(2):
                    nc.tensor.matmul(
                        pk[:, jt, :],
                        lhsT=w_qkv8[:, :, C + mb * 128: C + (mb + 1) * 128],
                        rhs=x8[:, :, (2 * ck + jt) * TT: (2 * ck + jt + 1) * TT],
                        start=True, stop=True, perf_mode=DR,
                    )
                evict(k8[:, mb, 2 * ck * TT: (2 * ck + 2) * TT]
                      .rearrange("p (j t) -> p j t", j=2),
                      pk, bias_qk[:, 2 + mb, :])

            def emit_v(cp):
                pv = psQ.tile([128, 2, TT], F32, tag="qk")
                for jt in range(2):
                    nc.tensor.matmul(
                        pv[:, jt, 0:C],
                        lhsT=x8[:, :, (2 * cp + jt) * 128: (2 * cp + jt + 1) * 128],
                        rhs=w_qkv8[:, :, 2 * C: 3 * C],
                        start=True, stop=True, perf_mode=DR,
                    )
                evict(vT[:, 2 * cp: 2 * cp + 2, :, 0:CH],
                      pv[:, :, 0:C].rearrange("p j (h c) -> p j h c", h=NH))

            # all production up front
            for mb in range(2):
                emit_q(mb)
            for ck in range(4):
                emit_k(0, ck)
            for ck in range(4):
                emit_k(1, ck)
            for cp in range(NPAIR):
                emit_v(cp)

            # ---------------- attention ----------------
            # deferred emission: AVs trail the exp by AV_LAG pair-slots, the
            # unit tail by TAIL_LAG (keeps the DVE-only reciprocal from
            # head-of-line-blocking the next unit's exps), proj a bit more.
            TAIL_LAG, PROJ_LAG = 10, 12
            gctr = [0]
            deferred = []

            def flush(now=None):
                deferred.sort(key=lambda x: x[0])
                while deferred and (now is None or deferred[0][0] <= now):
                    deferred.pop(0)[1]()

            def defer(delay, fn):
                deferred.append((gctr[0] + delay, fn))

            def make_av(av, pt_t, h, pi):
                def emit():
                    nc.tensor.matmul(
                        av,
                        lhsT=vT[:, 2 * pi: 2 * pi + 2, h, 0: CH + 1],
                        rhs=pt_t,
                        start=(pi == 0), stop=(pi == NPAIR - 1),
                        perf_mode=DR,
                    )
                return emit

            def make_tail(av, h, tt, ui, last):
                def emit():
                    tsl = slice(tt * TT, (tt + 1) * TT)
                    r_row = tmp.tile([1, TT], F32, tag="rrow")
                    nc.vector.reciprocal(r_row, av[CH: CH + 1, :])
                    rb = tmp.tile([64, TT], F32, tag="rbs")
                    if last:
                        # latency-critical: in-SBUF broadcast on Pool
                        nc.gpsimd.partition_broadcast(rb, r_row)
                    else:
                        # engine-free: bounce through DRAM via the DMA engines
                        nc.sync.dma_start(out=lscr[ui: ui + 1, :], in_=r_row)
                        nc.sync.dma_start(
                            out=rb, in_=lscr[ui: ui + 1, :].partition_broadcast(64))
                    if h % 2 == 0:
                        nc.vector.tensor_mul(
                            a_sb[0:64, h // 2, tsl], av[0:CH, :], rb)
                    else:
                        a_t = tmp.tile([64, TT], BF16, tag="abounce")
                        nc.vector.tensor_mul(a_t, av[0:CH, :], rb)
                        nc.sync.dma_start(
                            out=a_sb[64:128, h // 2, tsl], in_=a_t)
                return emit

            def make_proj(tt):
                def emit():
                    tsl = slice(tt * TT, (tt + 1) * TT)
                    pps = []
                    for mb in range(2):
                        pp = psV.tile([128, TT], F32, tag="av")
                        pps.append(pp)
                        for cb in range(2):
                            nc.tensor.matmul(
                                pp,
                                lhsT=w_proj[:, cb, mb * 128: (mb + 1) * 128],
                                rhs=a_sb[:, cb, tsl],
                                start=(cb == 0), stop=(cb == 1),
                            )
                    for mb in range(2):
                        nc.vector.scalar_tensor_tensor(
                            out=out_sb[:, mb, tsl], in0=pps[mb],
                            scalar=obias[:, mb, :], in1=xq[:, mb, tsl],
                            op0=mybir.AluOpType.add, op1=mybir.AluOpType.add,
                        )
                        nc.sync.dma_start(
                            out=out_ext[mb * 128: (mb + 1) * 128, tsl],
                            in_=out_sb[:, mb, tsl],
                        )
                return emit

            inject = {}

            for tt in range(NTT):
                tsl = slice(tt * TT, (tt + 1) * TT)
                if DBG and tt == 0:
                    nc.sync.dma_start(out=dbg_k8[:, :, :], in_=k8[:, :, 0:T])
                    nc.sync.dma_start(out=dbg_q8[:, :, :], in_=q8z[:, :, 0, :])
                    nc.sync.dma_start(out=dbg_vT[:, :, :, 0: CH + 1],
                                      in_=vT[:, :, :, 0: CH + 1])
                    nc.sync.dma_start(out=dbg_ga[:, :, :], in_=ga)
                for hn, h in enumerate((1, 3, 0, 2)):
                    ui = tt * NH + hn
                    mb, hp = h // 2, (h % 2) * 64
                    av = psV.tile([CH + 1, TT], F32, tag="av")
                    for pi in range(NPAIR):
                        flush(gctr[0])
                        if ui == 0:
                            for fn in inject.pop(pi, ()):
                                fn()
                        pt_t = ptp.tile([128, 2, TT], E5, tag="pt")
                        st = psQ.tile([128, 2, TT], F32, tag="qk")
                        for j2 in range(2):
                            j = 2 * pi + j2
                            nc.tensor.matmul(
                                st[:, j2, :],
                                lhsT=k8[hp: hp + 64, mb, j * SB: (j + 2) * SB]
                                .rearrange("p (two s) -> p two s", two=2),
                                rhs=q8z[hp: hp + 64, mb, :, tsl],
                                start=True, stop=True, perf_mode=DR,
                            )
                        eng = EXP_PATTERN[(pi + 5 * ui) % NPAIR]
                        if eng == "act":
                            nc.scalar.activation(
                                out=pt_t, in_=st,
                                func=mybir.ActivationFunctionType.Exp,
                                scale=1.0,
                            )
                        else:
                            nc.vector.tensor_scalar(
                                out=pt_t.bitcast(I8), in0=st,
                                scalar1=float(SCH_A), scalar2=float(SCH_B),
                                op0=mybir.AluOpType.mult,
                                op1=mybir.AluOpType.add,
                            )
                        if DBG and tt == 0 and h == 0:
                            nc.sync.dma_start(
                                out=dbg_pt[:, 2 * pi: 2 * pi + 2, :], in_=pt_t)
                        defer(AV_LAG, make_av(av, pt_t, h, pi))
                        gctr[0] += 1
                    last = (tt == NTT - 1) and (hn == NH - 1)
                    defer(TAIL_LAG, make_tail(av, h, tt, ui, last))
                defer(PROJ_LAG, make_proj(tt))
            flush()
            if DBG:
                nc.sync.dma_start(out=dbg_a[:, :, :], in_=a_sb)

    nc.compile()
    return nc


# ---------------- host side ----------------

def _prep_consts(qkv_w, qkv_b, proj_w, proj_b, norm_w, norm_b):
    qkv_w = np.asarray(qkv_w, np.float32)
    qkv_b = np.asarray(qkv_b, np.float32)
    proj_w = np.asarray(proj_w, np.float32)
    proj_b = np.asarray(proj_b, np.float32)
    # permute rows from per-head [q|k|v] interleave to [all q | all k | all v],
    # heads in order; fold the ch**-0.25 score scale into q and k
    perm = np.concatenate(
        [np.arange(NH)[:, None] * (3 * CH) + off + np.arange(CH)[None, :]
         for off in (0, CH, 2 * CH)]
    ).reshape(3 * C)
    wp = qkv_w[perm].copy()
    bp = qkv_b[perm].copy()
    wp[: 2 * C] *= SCALE
    bp[: 2 * C] *= SCALE
    # fold v bias into proj bias: pb' = proj_b + proj_w @ bv
    pb = proj_b + proj_w @ bp[2 * C:]
    gind = np.zeros((128, 16), np.float32)
    gindT = np.zeros((16, 128), np.float32)
    for p in range(128):
        gind[p, p // GSIZE] = 1.0 / GSIZE
        gindT[p // GSIZE, p] = 1.0
    def cb2(a):  # [256, ...] -> [128, 2, ...]
        return np.ascontiguousarray(a.reshape(2, 128, *a.shape[1:]).swapaxes(0, 1))
    consts = np.empty((128, 28), np.float32)
    consts[:, 0:16] = gind
    consts[:, 16:22] = bp.reshape(6, 128).T
    consts[:, 22:24] = pb.reshape(2, 128).T
    consts[:, 24:26] = np.asarray(norm_w, np.float32).reshape(2, 128).T
    consts[:, 26:28] = np.asarray(norm_b, np.float32).reshape(2, 128).T
    return {
        "wqkvT": cb2(wp.T).astype(ml_dtypes.bfloat16),
        "wprojT": cb2(proj_w.T).astype(ml_dtypes.bfloat16),
        "consts": consts,
        "gindT": gindT,
    }


def _make_in_maps(x, norm_w, norm_b, qkv_w, qkv_b, proj_w, proj_b):
    x = np.asarray(x, np.float32)
    consts = _prep_consts(qkv_w, qkv_b, proj_w, proj_b, norm_w, norm_b)
    xf = x.reshape(B, C, T)
    in_maps = []
    for core in range(NCORES):
        b, tq = core // 4, core % 4
        m = dict(consts)
        xb = xf[b].reshape(2, 128, T).swapaxes(0, 1)
        m["x8"] = np.ascontiguousarray(xb).astype(ml_dtypes.float8_e4m3).view(np.int8)
        xqv = np.ascontiguousarray(xb[:, :, tq * TQ: (tq + 1) * TQ])
        m["xq8"] = xqv.astype(ml_dtypes.float8_e4m3).view(np.int8)
        m["xq"] = xqv
        in_maps.append(m)
    return in_maps


def _assemble(results):
    out = np.empty((B, C, T), np.float32)
    for core in range(NCORES):
        b, tq = core // 4, core % 4
        out[b][:, tq * TQ: (tq + 1) * TQ] = results[core]["out"]
    return out.reshape(B, C, HS, WS)


def kernel(x, norm_w, norm_b, qkv_w, qkv_b, proj_w, proj_b):
    in_maps = _make_in_maps(x, norm_w, norm_b, qkv_w, qkv_b, proj_w, proj_b)
    nc = build_nc()
    results = bass2jax.run_bass_via_pjrt(nc, in_maps, n_cores=NCORES)
    return _assemble(results)


if __name__ == "__main__":
    rng = np.random.default_rng(0)
    out = kernel(
        rng.standard_normal((B, C, HS, WS), np.float32),
        np.ones(C, np.float32),
        np.zeros(C, np.float32),
        rng.standard_normal((3 * C, C), np.float32) * C**-0.5,
        rng.standard_normal(3 * C, np.float32) * 0.02,
        rng.standard_normal((C, C), np.float32) * C**-0.5,
        rng.standard_normal(C, np.float32) * 0.02,
    )
    print(out.shape, float(np.abs(out).max()))
